# revision 9
# baseline (speedup 1.0000x reference)
"""Disen-GCN (8-channel routing attention GNN) on 8 TRN2 NeuronCores.

Row-parallel sharding: core r owns node rows [r*512, (r+1)*512).
Per routing iteration:
  phase1: L[c][j, i_local] = z[c,j] . z[c,i]      (PE, K=64 row-tiled pairs)
  exp:    E[c] = exp(L[c])                        (ACT, PSUM->SBUF fp16)
  smax:   S = sum_c E[c]; Q = mask * 1/S          (DVE)
  R:      R[c] = E[c] * Q                         (DVE + GpSimd, in-place)
  phase3: agg^T[c][d, i] += znat[c][j,:]^T @ R[c] (PE, col-tiled pairs, PSUM acc)
  norm:   z = l2norm(z + agg) (PE blockdiag-sumsq + ACT ln/exp rsqrt + DVE)
  AllGather of new z rows (both layouts) via internal shared DRAM.
Final: out = concat_c(z) @ W_o + bias.
"""

import numpy as np
from contextlib import ExitStack

from concourse import bacc, bass, tile, mybir
from concourse.bass_utils import run_bass_kernel_spmd

F32 = mybir.dt.float32
F16 = mybir.dt.float16

N = 4096
C = 8
IN_DIM = 256
D = 64
OUT = 128
ITERS = 4
NCORES = 8
NL = N // NCORES          # 512 local rows
CD = C * D                # 512
NJT = N // 128            # 32 j-tiles
NPAIR = C // 2            # 4 channel-pair tiles
AF = mybir.ActivationFunctionType
RG = [list(range(NCORES))]
N_GP_MUL = 3              # R-muls offloaded to GpSimd per j-tile


def _build_nc():
    nc = bacc.Bacc(
        "TRN2", target_bir_lowering=False, debug=False, num_devices=NCORES
    )
    featT = nc.dram_tensor("featT", [IN_DIM, NL], F16, kind="ExternalInput").ap()
    wall = nc.dram_tensor("wall", [IN_DIM, CD], F16, kind="ExternalInput").ap()
    bflat = nc.dram_tensor("bflat", [1, CD], F16, kind="ExternalInput").ap()
    maskT = nc.dram_tensor("maskT", [N, NL], F16, kind="ExternalInput").ap()
    wo = nc.dram_tensor("wo", [CD, OUT], F16, kind="ExternalInput").ap()
    biasd = nc.dram_tensor("biasd", [1, OUT], F16, kind="ExternalInput").ap()
    ident = nc.dram_tensor("ident", [128, 128], F16, kind="ExternalInput").ap()
    blkd = nc.dram_tensor("blkd", [128, NPAIR * 8], F16, kind="ExternalInput").ap()
    seld = nc.dram_tensor("seld", [8, NPAIR * 128], F16, kind="ExternalInput").ap()
    onesd = nc.dram_tensor("onesd", [1, 128], F16, kind="ExternalInput").ap()
    outd = nc.dram_tensor("outd", [NL, OUT], F32, kind="ExternalOutput").ap()

    with tile.TileContext(nc) as tc:
        _body(nc, tc, featT, wall, bflat, maskT, wo, biasd, ident, blkd, seld,
              onesd, outd)
    nc.compile()
    return nc


def _body(nc, tc, featT, wall, bflat, maskT, wo, biasd, ident, blkd, seld,
          onesd, outd):
    ctx = ExitStack()
    const = ctx.enter_context(tc.tile_pool(name="const", bufs=1))
    big = ctx.enter_context(tc.tile_pool(name="big", bufs=1))
    work = ctx.enter_context(tc.tile_pool(name="work", bufs=1))
    psum = ctx.enter_context(tc.tile_pool(name="psum", bufs=1, space="PSUM"))
    dram = ctx.enter_context(tc.tile_pool(name="dram", bufs=1, space="DRAM"))

    def loadc(dr_ap, shape, name):
        dst = const.tile(shape, F16, tag=name, bufs=1, name=name)
        nc.sync.dma_start(out=dst, in_=dr_ap)
        return dst

    # ---- constants / weights (fp16 already on host) ----
    ident16 = loadc(ident, [128, 128], "ident16")
    blkd16 = loadc(blkd, [128, NPAIR * 8], "blkd16")
    sel16 = loadc(seld, [8, NPAIR * 128], "sel16")
    ones16 = loadc(onesd, [1, 128], "ones16")
    b16 = loadc(bflat, [1, CD], "b16")
    bias16 = loadc(biasd, [1, OUT], "bias16")
    zeros16 = const.tile([1, NL], F16, tag="zeros16", bufs=1, name="zeros16")
    nc.vector.memset(zeros16, 0.0)

    featT16 = const.tile([128, 2 * NL], F16, tag="featT16", bufs=1, name="featT16")
    nc.sync.dma_start(
        out=featT16.rearrange("p (k i) -> p k i", k=2),
        in_=featT.rearrange("(k p) i -> p k i", p=128))
    w016 = const.tile([128, 2 * CD], F16, tag="w016", bufs=1, name="w016")
    nc.sync.dma_start(
        out=w016.rearrange("p (k i) -> p k i", k=2),
        in_=wall.rearrange("(k p) i -> p k i", p=128))
    wo16 = const.tile([128, 4 * OUT], F16, tag="wo16", bufs=1, name="wo16")
    nc.sync.dma_start(
        out=wo16.rearrange("p (k i) -> p k i", k=4),
        in_=wo.rearrange("(k p) i -> p k i", p=128))

    # ---- resident mask (fp16): mask16[:, jt*512 + i] = adj[i_global, j] ----
    mask16 = big.tile([128, NJT * NL], F16, tag="mask16", bufs=1, name="mask16")
    nc.sync.dma_start(
        out=mask16.rearrange("p (j i) -> p j i", j=NJT),
        in_=maskT.rearrange("(j p) i -> p j i", p=128))

    # ---- resident full z, both layouts (fp16) ----
    zT16 = [big.tile([128, N], F16, tag=f"zT{t}", bufs=1, name=f"zT16_{t}")
            for t in range(NPAIR)]
    znat16 = big.tile([128, NJT * CD], F16, tag="znat16", bufs=1, name="znat16")

    def normalize_and_rows(zpre, it):
        """zpre: 4 SBUF fp16 tiles [128, NL] (z_T rows layout, pre-norm).
        Returns (zrows, natrows): l2-normalized rows in both layouts."""
        nrm = psum.tile([8, NL], F32, tag="L", bufs=4, name=f"nrm_{it}")
        for t in range(NPAIR):
            sq = work.tile([128, NL], F16, tag="sq", bufs=2, name=f"sq_{it}_{t}")
            nc.vector.tensor_mul(out=sq, in0=zpre[t], in1=zpre[t])
            nc.tensor.matmul(out=nrm, lhsT=blkd16[:, t * 8:(t + 1) * 8], rhs=sq,
                             start=(t == 0), stop=(t == NPAIR - 1))
        nrmc = work.tile([8, NL], F32, tag="nrmc", bufs=2, name=f"nrmc_{it}")
        nc.vector.tensor_scalar_max(out=nrmc, in0=nrm, scalar1=1e-12)
        lnn = work.tile([8, NL], F32, tag="lnn", bufs=2, name=f"lnn_{it}")
        nc.scalar.activation(out=lnn, in_=nrmc, func=AF.Ln)
        rsq = work.tile([8, NL], F16, tag="rsq", bufs=2, name=f"rsq_{it}")
        nc.scalar.activation(out=rsq, in_=lnn, func=AF.Exp, scale=-0.5)
        zrows = []
        for t in range(NPAIR):
            bc = psum.tile([128, NL], F32, tag="L", bufs=4, name=f"bc_{it}_{t}")
            nc.tensor.matmul(out=bc, lhsT=sel16[:, t * 128:(t + 1) * 128],
                             rhs=rsq, start=True, stop=True)
            zr = work.tile([128, NL], F16, tag="zrows", bufs=8,
                           name=f"zrows_{it}_{t}")
            nc.vector.tensor_mul(out=zr, in0=zpre[t], in1=bc)
            zrows.append(zr)
        natrows = [work.tile([128, CD], F16, tag="natrows", bufs=8,
                             name=f"natr_{it}_{ib}") for ib in range(4)]
        for t in range(NPAIR):
            for ib in range(4):
                tp = psum.tile([128, 128], F16, tag="L", bufs=4,
                               name=f"tp_{it}_{t}_{ib}")
                nc.tensor.transpose(out=tp,
                                    in_=zrows[t][:, ib * 128:(ib + 1) * 128],
                                    identity=ident16)
                nc.vector.tensor_copy(
                    out=natrows[ib][:, t * 128:(t + 1) * 128], in_=tp)
        return zrows, natrows

    def ship(zrows, natrows, it):
        """AllGather both layouts of the local z rows; refill zT16/znat16."""
        ag_in = dram.tile([2 * NL, CD], F16, tag="agin", bufs=2,
                          name=f"agin_{it}")
        for ib in range(4):
            nc.sync.dma_start(out=ag_in[ib * 128:(ib + 1) * 128, :],
                              in_=natrows[ib])
        for t in range(NPAIR):
            nc.sync.dma_start(out=ag_in[NL + t * 128:NL + (t + 1) * 128, :],
                              in_=zrows[t])
        ag_out = dram.tile([NCORES * 2 * NL, CD], F16, tag="agout", bufs=2,
                           addr_space="Shared", name=f"agout_{it}")
        nc.gpsimd.collective_compute(
            "AllGather", mybir.AluOpType.bypass, replica_groups=RG,
            ins=[ag_in.opt()], outs=[ag_out.opt()])
        # batched readback: nat chunks (one DMA per rank)
        ag_view = ag_out.rearrange("(r q) d -> r q d", r=NCORES)
        for r in range(NCORES):
            nc.sync.dma_start(
                out=znat16[:, r * 4 * CD:(r + 1) * 4 * CD].rearrange(
                    "p (j d) -> p j d", j=4),
                in_=ag_view[r, 0:NL, :].rearrange("(j p) d -> p j d", p=128))
        # batched readback: zT chunks (one DMA per pair tile, all ranks)
        for t in range(NPAIR):
            nc.sync.dma_start(
                out=zT16[t].rearrange("p (r i) -> p r i", r=NCORES),
                in_=ag_view[:, NL + t * 128:NL + (t + 1) * 128, :].rearrange(
                    "r p d -> p r d"))

    # ================= phase 0: z0 = l2norm(features @ W + b) =================
    z0n = []
    for ib in range(4):
        zp = psum.tile([128, CD], F32, tag="L", bufs=4, name=f"zp_{ib}")
        for kt in range(2):
            nc.tensor.matmul(
                out=zp,
                lhsT=featT16[:, kt * NL + ib * 128:kt * NL + (ib + 1) * 128],
                rhs=w016[:, kt * CD:(kt + 1) * CD],
                start=(kt == 0), stop=False)
        nc.tensor.matmul(out=zp, lhsT=ones16, rhs=b16, start=False, stop=True)
        zn = work.tile([128, CD], F16, tag="z0n", bufs=4, name=f"z0n_{ib}")
        nc.vector.tensor_copy(out=zn, in_=zp)
        z0n.append(zn)
    zpre0 = []
    for t in range(NPAIR):
        zp_t = work.tile([128, NL], F16, tag="zpre0", bufs=8, name=f"zpre0_{t}")
        for ib in range(4):
            tp = psum.tile([128, 128], F16, tag="L", bufs=4, name=f"tp0_{t}_{ib}")
            nc.tensor.transpose(out=tp, in_=z0n[ib][:, t * 128:(t + 1) * 128],
                                identity=ident16)
            nc.vector.tensor_copy(out=zp_t[:, ib * 128:(ib + 1) * 128], in_=tp)
        zpre0.append(zp_t)
    zrows, natrows = normalize_and_rows(zpre0, it=-1)
    ship(zrows, natrows, it=-1)

    # ================= routing iterations =================
    for it in range(ITERS):
        agg = [psum.tile([128, NL], F32, tag="agg", bufs=4, name=f"agg_{it}_{t}")
               for t in range(NPAIR)]
        for t in range(NPAIR):
            # zero-fill the whole bank once so both col-tiled halves can
            # accumulate with start=False (start clears the full bank)
            nc.tensor.matmul(out=agg[t], lhsT=ones16, rhs=zeros16,
                             start=True, stop=False)
        for jt in range(NJT):
            Es = []
            for c in range(C):
                t, h = c // 2, c % 2
                L = psum.tile([128, NL], F32, tag="L", bufs=4,
                              name=f"L_{it}_{jt}_{c}")
                nc.tensor.matmul(
                    out=L,
                    lhsT=zT16[t][h * 64:(h + 1) * 64, jt * 128:(jt + 1) * 128],
                    rhs=zrows[t][h * 64:(h + 1) * 64, :],
                    start=True, stop=True, tile_position=(h * 64, 0))
                E = work.tile([128, NL], F16, tag="E", bufs=12,
                              name=f"E_{it}_{jt}_{c}")
                nc.scalar.activation(out=E, in_=L, func=AF.Exp)
                Es.append(E)
            # channel-softmax denominator (f16 tree sum on DVE)
            s2 = []
            for k in range(4):
                s = work.tile([128, NL], F16, tag="s2", bufs=5,
                              name=f"s2_{it}_{jt}_{k}")
                nc.vector.tensor_add(out=s, in0=Es[2 * k], in1=Es[2 * k + 1])
                s2.append(s)
            s4a = work.tile([128, NL], F16, tag="s4", bufs=3,
                            name=f"s4a_{it}_{jt}")
            nc.vector.tensor_add(out=s4a, in0=s2[0], in1=s2[1])
            s4b = work.tile([128, NL], F16, tag="s4b", bufs=3,
                            name=f"s4b_{it}_{jt}")
            nc.vector.tensor_add(out=s4b, in0=s2[2], in1=s2[3])
            S16 = work.tile([128, NL], F16, tag="S16", bufs=3,
                            name=f"S16_{it}_{jt}")
            nc.vector.tensor_add(out=S16, in0=s4a, in1=s4b)
            Sf = work.tile([128, NL], F32, tag="Sf", bufs=2, name=f"Sf_{it}_{jt}")
            nc.vector.tensor_copy(out=Sf, in_=S16)
            Sinv = work.tile([128, NL], F32, tag="Sinv", bufs=2,
                             name=f"Sinv_{it}_{jt}")
            nc.vector.reciprocal_approx_fast(out=Sinv, in_=Sf)
            Q = work.tile([128, NL], F16, tag="Q", bufs=3, name=f"Q_{it}_{jt}")
            nc.vector.tensor_mul(out=Q, in0=mask16[:, jt * NL:(jt + 1) * NL],
                                 in1=Sinv)
            for c in range(C):
                eng = nc.gpsimd if c < N_GP_MUL else nc.vector
                eng.tensor_mul(out=Es[c], in0=Es[c], in1=Q)
            # aggregation: agg^T[c][d, i] += znat[c,jt]^T @ R[c]
            for c in range(C):
                t, h = c // 2, c % 2
                nc.tensor.matmul(
                    out=agg[t][h * 64:(h + 1) * 64, :],
                    lhsT=znat16[:, jt * CD + c * 64:jt * CD + (c + 1) * 64],
                    rhs=Es[c],
                    start=False, stop=False,
                    tile_position=(0, h * 64))
        for t in range(NPAIR):
            # full-bank dummy stop closes the accumulation group (adds zeros)
            nc.tensor.matmul(out=agg[t], lhsT=ones16, rhs=zeros16,
                             start=False, stop=True)
        # residual + renorm
        zpre = []
        for t in range(NPAIR):
            zq = work.tile([128, NL], F16, tag="zpre0", bufs=8,
                           name=f"zpre_{it}_{t}")
            nc.vector.tensor_add(out=zq, in0=zrows[t], in1=agg[t])
            zpre.append(zq)
        zrows, natrows = normalize_and_rows(zpre, it=it)
        if it < ITERS - 1:
            ship(zrows, natrows, it=it)

    # ================= output: h @ W_o + bias =================
    for ib in range(4):
        op = psum.tile([128, OUT], F32, tag="L", bufs=4, name=f"op_{ib}")
        for kt in range(4):
            nc.tensor.matmul(out=op,
                             lhsT=zrows[kt][:, ib * 128:(ib + 1) * 128],
                             rhs=wo16[:, kt * OUT:(kt + 1) * OUT],
                             start=(kt == 0), stop=False)
        nc.tensor.matmul(out=op, lhsT=ones16, rhs=bias16, start=False, stop=True)
        ot = work.tile([128, OUT], F32, tag="ot", bufs=2, name=f"ot_{ib}")
        nc.vector.tensor_copy(out=ot, in_=op)
        nc.sync.dma_start(out=outd[ib * 128:(ib + 1) * 128, :], in_=ot)

    ctx.close()


def _make_in_maps(features, adj, W, b, W_o, bias):
    features = np.asarray(features, dtype=np.float32)
    adj = np.asarray(adj, dtype=np.float32)
    W = np.asarray(W, dtype=np.float32)
    b = np.asarray(b, dtype=np.float32)
    W_o = np.asarray(W_o, dtype=np.float32)
    bias = np.asarray(bias, dtype=np.float32)

    f16 = np.float16
    wall = np.ascontiguousarray(
        W.transpose(1, 0, 2).reshape(IN_DIM, CD)).astype(f16)
    bflat = np.ascontiguousarray(b.reshape(1, CD)).astype(f16)
    ident = np.eye(128, dtype=f16)
    blkd = np.zeros((128, NPAIR * 8), dtype=f16)
    seld = np.zeros((8, NPAIR * 128), dtype=f16)
    for t in range(NPAIR):
        for h in range(2):
            c = 2 * t + h
            blkd[h * 64:(h + 1) * 64, t * 8 + c] = 1.0
            seld[c, t * 128 + h * 64:t * 128 + (h + 1) * 64] = 1.0
    onesd = np.ones((1, 128), dtype=f16)
    wo16 = W_o.astype(f16)
    bias16 = bias.reshape(1, OUT).astype(f16)

    in_maps = []
    for r in range(NCORES):
        rows = slice(r * NL, (r + 1) * NL)
        in_maps.append({
            "featT": np.ascontiguousarray(features[rows].T).astype(f16),
            "wall": wall,
            "bflat": bflat,
            "maskT": np.ascontiguousarray(adj[rows].T).astype(f16),
            "wo": wo16,
            "biasd": bias16,
            "ident": ident,
            "blkd": blkd,
            "seld": seld,
            "onesd": onesd,
        })
    return in_maps


_NC_CACHE = []


def _get_nc():
    if not _NC_CACHE:
        _NC_CACHE.append(_build_nc())
    return _NC_CACHE[0]


def run(inputs, trace=False, **kwargs):
    nc = _get_nc()
    in_maps = _make_in_maps(**inputs)
    res = run_bass_kernel_spmd(nc, in_maps, core_ids=list(range(NCORES)),
                               trace=trace, **kwargs)
    out = np.concatenate([res.results[r]["outd"] for r in range(NCORES)],
                         axis=0).astype(np.float32)
    return out, res


def kernel(features, adj, W, b, W_o, bias):
    out, _ = run(dict(features=features, adj=adj, W=W, b=b, W_o=W_o, bias=bias))
    return out
